# revision 48
# baseline (speedup 1.0000x reference)
"""Trainium2 Bass kernel (v10) for LocalDynamicGraph edge-feature construction.

Per batch element b (one NeuronCore each, data-parallel over B=8):
    out[b, n, c, k] = x[b, idx[b,n,k], c] - x[b, n, c]   for c < 64
    out[b, n, c, k] = x[b, n, c - 64]                    for c >= 64

v10 = v8's SWDGE-gather pipeline, stripped to its engine-bus floor.
Trace analysis of v8 showed the SDMA engines are the bottleneck: per
128B gather descriptor ~13ns of engine-bus time (desc fetch + HBM read
+ SBUF write) plus 64 MiB of output writes at ~25 GB/s/engine; Pool
desc-gen itself is fast (~0.1us/call; its long trace slices were ring
backpressure).  So v10 removes everything else from the bus and the
critical path:

  - xbf (bf16 table, 256B-stride rows) is host-prepped and loaded
    directly (2 MB) instead of being staged x->SBUF->cast->HBM; the
    fp32 warm-phase gathers (256B descriptors) are gone - all 64
    blocks gather bf16/2048-idx/per-descriptor-packet calls.
  - centers are host-laid-out contiguously (xcb [128, 64*64] fp32,
    partition = point-in-block) so the 2 MB load is 16KB/partition
    contiguous instead of 8192 x 256B descriptors.
  - bigger SWDGE descriptor rings (128 KB scratch) so Pool never
    stalls the queues.
  - DVE computes (neighbor - center) via a strided bf16 view, ACT
    broadcast-copies the fp32 center half (bit-exact), each block is
    written back as one fully contiguous 1 MB DMA.
"""

import numpy as np

import concourse.bacc as bacc
import concourse.mybir as mybir
from concourse.tile import TileContext
from concourse.bass_utils import run_bass_kernel_spmd

# Problem constants (hardcoded per contest contract).
B = 8
N = 8192
C = 64
K = 16
P = 128              # partitions / points per output tile
NBLK = N // P        # 64 point-blocks per core
NQ = 4               # SWDGE queues (ucode max)
GS = K * P           # indices per gather call (one block = 2048)
XPAD = 128           # bf16 row padded to 128 elements = 256B stride
COLS = NBLK * GS // 16   # idxw columns = 8192

_NC_CACHE = {}


def _dma_gather_raw(gp, out_ap, in_ap, idxs_ap, num_idxs, num_idxs_reg,
                    elem_size, elem_step, queue_num, single_packet=True):
    """bass.dma_gather minus the elem_size%256B assert. The SWDGE ucode
    only needs the source stride (elem_step bytes) to be a multiple of
    256; the per-index payload is a plain descriptor length."""
    dtsize = mybir.dt.size(in_ap.dtype)
    assert in_ap.dtype == out_ap.dtype
    assert idxs_ap.dtype == mybir.dt.int16
    stride_bytes = elem_step * dtsize
    assert stride_bytes % 256 == 0
    stride_256 = stride_bytes // 256
    assert 0 < stride_256 < 256
    assert in_ap.ap[0][0] == elem_step
    assert in_ap.ap[-1][1] == elem_size
    assert out_ap.ap[-1][1] == elem_size
    assert out_ap.ap[0][1] * out_ap.ap[1][1] == num_idxs
    _in_ap = gp.lower_ap_dma(in_ap, for_custom_bir_dma=True)
    _idxs_ap = gp.lower_ap(idxs_ap)
    _out_ap = gp.lower_ap(out_ap)
    return gp.add_instruction(
        mybir.InstDMAGatherAnt(
            name=gp.bass.get_next_instruction_name(),
            ins=[
                *_in_ap,
                _idxs_ap,
                gp.lower_val_access(gp.to_reg(num_idxs_reg)),
            ],
            outs=[_out_ap],
            transpose=False,
            num_idxs=num_idxs,
            elem_size=elem_size,
            stride_bytes_256=stride_256,
            gen_mode=0,
            single_packet=single_packet,
            queue_num=queue_num,
            sbuf_tokens_per_rank=0,
            sbuf_free_dim_per_rank=0,
            sbuf_free_dim_pad_per_rank=0,
            sbuf_byte_offset=0,
        )
    )


def build_nc():
    nc = bacc.Bacc(
        "TRN2",
        target_bir_lowering=False,
        dynamic_dma_scratch_size=98304,
        num_swdge_queues=NQ,
    )
    xbf = nc.dram_tensor(
        "xbf", [N, XPAD], mybir.dt.bfloat16, kind="ExternalInput"
    ).ap()
    xcb_d = nc.dram_tensor(
        "xcb", [P, NBLK * C], mybir.dt.float32, kind="ExternalInput"
    ).ap()
    idxw_d = nc.dram_tensor(
        "idxw", [P, COLS], mybir.dt.int16, kind="ExternalInput"
    ).ap()
    out = nc.dram_tensor(
        "out", [N, 2 * C * K], mybir.dt.float32, kind="ExternalOutput"
    ).ap()
    # Pair view: blocks 2j,2j+1 are 2MB contiguous in HBM; one DMA per
    # pair halves the per-DMA completion overhead (the final sem
    # descriptor waits ~2us for HBM write acks at load). 2:1 is the
    # sweet spot: per-block DMAs +6us, 4:1 quads +38us (lumpy stream,
    # 2-deep pool).
    OB = 2
    out_quads = out.rearrange("(j h p) f -> j p h f", h=OB, p=P)

    with TileContext(nc) as tc:
        with (
            tc.tile_pool(name="const", bufs=1) as cpool,
            tc.tile_pool(name="gat", bufs=8) as gpool,
            tc.tile_pool(name="outp", bufs=4) as opool,
        ):
            # Wrapped indices, replicated across all 16-partition groups.
            # Loaded per-call-chunk so early gathers aren't gated on 2MB.
            # Inputs ride the ACT HWDGE ring so the SP ring carries only
            # output writes from t=0. (Pacing input loads into the block
            # loop measured worse: 341us vs 320us.)
            # One tile PER idx chunk: with a single [P, COLS] tile the
            # gathers' dep covered the whole tile, so the first gather
            # waited for all 16 chunk loads (first gather at 18us, exactly
            # after the 17th serial 0.6us DMA setup on the ACT sequencer).
            IDX_CHUNKS = 16
            ccols = COLS // IDX_CHUNKS
            BLKS_PER_CHUNK = NBLK // IDX_CHUNKS
            ichunks = [
                cpool.tile([P, ccols], mybir.dt.int16, name=f"ichunk{ch}")
                for ch in range(IDX_CHUNKS)
            ]

            def load_idx_chunk(ch):
                # ACT ring: on the SP ring these sit ahead of the output
                # writes in FIFO order and measured 32us slower.
                nc.scalar.dma_start(
                    ichunks[ch][:], idxw_d[:, ch * ccols : (ch + 1) * ccols]
                )

            # Centers, host-laid-out [p, nb*C + c]; contiguous 16KB/partition.
            # Loaded right after idx chunk 0 so ACT's center copies (which
            # need only xcb) start ~10us earlier.
            xcb_sb = cpool.tile([P, NBLK * C], mybir.dt.float32)
            load_idx_chunk(0)
            nc.scalar.dma_start(xcb_sb[:], xcb_d)  # ACT ring: xcb only
            for ch in range(1, IDX_CHUNKS):
                load_idx_chunk(ch)

            xbf_src = xbf[:, 0:C]  # ap [(XPAD, N), (1, C)]: 256B stride, 128B payload
            # One 2048-index call per block: the ~1us fixed SWDGE cost per
            # call makes smaller sub-calls Pool-bound (256x512 measured
            # 330us Pool busy); queue rotation keeps all 4 rings stocked.
            nreg = nc.gpsimd.to_reg(GS)
            GSUB = GS // NQ
            sreg = nc.gpsimd.to_reg(GSUB)
            for nb in range(NBLK):
                gt = gpool.tile([P, K * C], mybir.dt.bfloat16)
                icols = ichunks[nb // BLKS_PER_CHUNK][
                    :,
                    (nb % BLKS_PER_CHUNK) * (GS // 16) :
                    (nb % BLKS_PER_CHUNK + 1) * (GS // 16),
                ]
                if nb < NQ:
                    # Ramp: one 2048-idx call trickles through ~13 serial
                    # ring-refill cycles (~1.2us each) on a single queue,
                    # so block 0 only completes at ~38us. Spreading the
                    # first blocks over all 4 queues (4x512) parallelizes
                    # the refills and starts the output stream ~10us
                    # earlier. Steady state keeps 1 call/block (the ~1us
                    # fixed cost per call makes full splitting Pool-bound).
                    kz = GSUB // P  # k-slices per sub-call
                    for s in range(NQ):
                        _dma_gather_raw(
                            nc.gpsimd,
                            out_ap=gt[
                                :, s * kz * C : (s + 1) * kz * C
                            ].rearrange("p (g c) -> p g c", c=C),
                            in_ap=xbf_src,
                            idxs_ap=icols[
                                :, s * (GSUB // 16) : (s + 1) * (GSUB // 16)
                            ],
                            num_idxs=GSUB,
                            num_idxs_reg=sreg,
                            elem_size=C,
                            elem_step=XPAD,
                            queue_num=s,
                            single_packet=False,
                        )
                else:
                    _dma_gather_raw(
                        nc.gpsimd,
                        out_ap=gt[:].rearrange("p (g c) -> p g c", c=C),
                        in_ap=xbf_src,
                        idxs_ap=icols,
                        num_idxs=GS,
                        num_idxs_reg=nreg,
                        elem_size=C,
                        elem_step=XPAD,
                        queue_num=nb % NQ,
                        # >64-desc concatenated packets hang the SDMA.
                        single_packet=False,
                    )
                # Full-row output tiles: the 8KB HBM row is the descriptor
                # coalescing unit (half-row writes fragment to 4KB descs and
                # measured 24us slower). OB blocks share one tile and DMA.
                if nb % OB == 0:
                    ot = opool.tile([P, OB * 2 * C * K], mybir.dt.float32)
                half = ot[:, (nb % OB) * 2 * C * K : (nb % OB + 1) * 2 * C * K]
                neigh = (
                    gt[:].rearrange("p (r c) -> p r c", c=C).transpose([0, 2, 1])
                )  # (P, C, K) strided view of the k-major gathered rows
                centr = xcb_sb[:, nb * C : (nb + 1) * C]  # (P, C)
                centr_b = centr.unsqueeze(2).broadcast_to([P, C, K])
                dst1 = half[:, 0 : C * K].rearrange("p (c k) -> p c k", k=K)
                dst2 = half[:, C * K : 2 * C * K].rearrange("p (c k) -> p c k", k=K)
                nc.vector.tensor_sub(dst1, neigh, centr_b)
                nc.scalar.copy(dst2, centr_b)
                if nb % OB == OB - 1:
                    # Alternate pair writes over the two HWDGE rings.
                    oeng = nc.sync if (nb // OB) % 2 == 0 else nc.scalar
                    oeng.dma_start(
                        out_quads[nb // OB],
                        ot[:].rearrange("p (h f) -> p h f", h=OB),
                    )
    nc.compile()
    return nc


def get_nc():
    if "nc" not in _NC_CACHE:
        _NC_CACHE["nc"] = build_nc()
    return _NC_CACHE["nc"]


def _prep_inputs(x: np.ndarray, idx: np.ndarray):
    """Host-side layout prep (pure layout/precision, no gather/arith).

    xbf:  (B, N, 128) bf16 - x rows padded to 256B stride.
    xcb:  (B, 128, NBLK*C) fp32 - centers, partition = point-in-block.
    idxw: (B, 128, COLS) int16 - per-block k-major (l = k*128+p) gather
          lists, 16-wrapped (l%16 -> partition row, l//16 -> column) and
          replicated across the eight 16-partition GPSIMD core groups.
    """
    x = np.ascontiguousarray(np.asarray(x, dtype=np.float32))
    bf16 = mybir.dt.np(mybir.dt.bfloat16)
    xbf = np.zeros((B, N, XPAD), dtype=bf16)
    xbf[:, :, 0:C] = x.astype(bf16)
    xcb = np.ascontiguousarray(
        x.reshape(B, NBLK, P, C).transpose(0, 2, 1, 3).reshape(B, P, NBLK * C)
    )
    idx16 = np.asarray(idx).astype(np.int16).reshape(B, NBLK, P, K)
    flat = idx16.transpose(0, 1, 3, 2).reshape(B, NBLK, K * P)  # l = k*128+p
    wrapped = (
        flat.reshape(B, NBLK, GS // 16, 16)
        .transpose(0, 3, 1, 2)
        .reshape(B, 16, COLS)
    )
    rep = np.broadcast_to(wrapped[:, None, :, :], (B, 8, 16, COLS))
    idxw = np.ascontiguousarray(rep.reshape(B, P, COLS))
    return xbf, xcb, idxw


def run_on_hw(x: np.ndarray, idx: np.ndarray, **spmd_kwargs):
    """Run the bass kernel on 8 NeuronCores. Returns (out, BassKernelResults)."""
    xbf, xcb, idxw = _prep_inputs(x, idx)
    in_maps = [
        {"xbf": xbf[b], "xcb": xcb[b], "idxw": idxw[b]} for b in range(B)
    ]
    res = run_bass_kernel_spmd(
        get_nc(), in_maps, core_ids=list(range(B)), **spmd_kwargs
    )
    out = np.stack([r["out"].reshape(N, 2 * C, K) for r in res.results])
    return out, res


def kernel(x: np.ndarray, idx: np.ndarray) -> np.ndarray:
    out, _ = run_on_hw(x, idx)
    return out
